# revision 2
# baseline (speedup 1.0000x reference)
"""Trainium2 Bass kernel v4 for nn_Attention_36137854828870.

Multi-head causal attention with rotary embeddings:
  y = softmax((rope(x@wq) @ rope(x@wk)^T)/sqrt(hd) + causal) @ (x@wv) @ wo

Sharding (8 cores): data-parallel over batch (4) x tensor-parallel over
heads (2 groups of 8).  Core c handles batch c//2, head group c%2; host
sums the two partial (S, D) outputs per batch.

Everything runs in fp16 on the PE (1 cycle/column; fp8 DoubleRow would
halve this but is TRN3-only silicon).  Per-core pipeline, all on-chip:

  - Projections stream in 4 s-chunks of 512.  wq/wk columns are
    host-permuted ("permA2") so rope pairs are lane-aligned: head h of
    the group lives in partition band 32*(h%4)..+32 of partition-tile
    pair (2*(h//4), 2*(h//4)+1), even rope-dims in the even tile, odd
    dims in the odd tile.  Rope runs on DVE in fp16 2x mode.
  - A small SBUF->SBUF DMA shuffle (8 DMAs per tensor-chunk, 64 total)
    rearranges rope output into head-contiguous K16/Q16 (head h = 64
    rows at 64*(h%2), tile h//2, dims in original order).  No DRAM
    staging roundtrip, no gpsimd SWDGE storm.
  - Scores: fp16 64-partition matmuls, keys on PSUM partitions, q on
    free dim, emitted per key-tile pair with per-tile causal trimming.
  - exp on ScalarE once per key-tile pair ([128, 2, cols] PSUM->SBUF),
    scale=1/sqrt(hd) folded in, no max-subtraction (|scores| <= ~4).
    Causal triangles masked with one 2-D-pattern gpsimd.affine_select
    per diagonal pair, off the exp->PV critical path.
  - PV: fp16 per key-tile, accumulating [65, cols] PSUM (64 head dims
    + a ones-column denominator row riding in V).
  - Deferred normalization: reciprocal of the denominator row, 0-stride
    broadcast DMA, fused multiply-evict (even heads) or the
    baseline-proven shift-copy + in-place multiply (odd heads).
  - wo in fp16 with q on PSUM partitions; fp16 y out (host upcasts and
    sums the two per-batch partials).

Emission interleaves chunk-c projections with q-block c-1 attention so
the ScalarE exp stream never starves behind PE's projection stream; wo
matmuls are sprinkled one at a time between q-block-3 attention pairs
to fill PE while ScalarE finishes the last exps.
"""

import sys

sys.path.insert(0, "/opt/trn_rl_repo")

import numpy as np

import concourse.bass as bass
import concourse.mybir as mybir
import concourse.tile as tile
from concourse import bacc
from concourse.bass_utils import run_bass_kernel_spmd

B, S, D = 4, 2048, 1024
H, HD = 16, 64
P = 128
NCORES = 8
HPC = H // 2          # heads per core
DG = HPC * HD         # 512
NKT = D // P          # 8
NDT = DG // P         # 4
CW = 512              # s-chunk / q-block width
NSC = S // CW         # 4
NST = S // P          # 16 key tiles
F32 = mybir.dt.float32
F16 = mybir.dt.float16
F8 = mybir.dt.float8e4
AF = mybir.ActivationFunctionType
SWI = mybir.MatmulPerfMode.DoubleRowSwInterleave
WS = 16.0             # host scale on wq/wk (fp8 denormal avoidance)
NKP = NKT // 2        # 4 contraction k-tile pairs

_PROGRAM = None


def _build_program():
    nc = bacc.Bacc("TRN2", target_bir_lowering=False, debug=False)

    # all inputs pre-packed on host into SBUF tile layout (fully
    # contiguous per DMA -> no small-descriptor penalty)
    xT_d = nc.dram_tensor("xT", [P, NSC, NKT, CW], F16, kind="ExternalInput")
    # fp8 x variants for the residual projection (a: x8, b: (x/16)8, c: (4dx)8)
    x8_d = nc.dram_tensor("x8", [P, NSC, NKT, CW], F8, kind="ExternalInput")
    xb_d = nc.dram_tensor("xb", [P, NSC, NKT, CW], F8, kind="ExternalInput")
    xc8_d = nc.dram_tensor("xc8", [P, NSC, NKT, CW], F8, kind="ExternalInput")
    # interleaved fp8 weight variants [P, NKP, NDT, 256]
    wq8_d = [nc.dram_tensor(f"wq8{v}", [P, NKP, NDT, 2 * P], F8,
                            kind="ExternalInput") for v in range(3)]
    wk8_d = [nc.dram_tensor(f"wk8{v}", [P, NKP, NDT, 2 * P], F8,
                            kind="ExternalInput") for v in range(3)]
    wv_d = nc.dram_tensor("wv", [P, NKT, DG], F16, kind="ExternalInput")
    wvr_d = nc.dram_tensor("wvr", [P, NKT, DG], F16, kind="ExternalInput")
    wo_d = nc.dram_tensor("wo", [P, NDT, D], F16, kind="ExternalInput")
    cos_d = nc.dram_tensor("cost", [P, S], F16, kind="ExternalInput")
    sin_d = nc.dram_tensor("sint", [P, S], F16, kind="ExternalInput")
    y_d = nc.dram_tensor("y", [S, D], F16, kind="ExternalOutput")

    xT_v = xT_d.ap()

    with tile.TileContext(nc) as tc:
        with tc.tile_pool(name="const", bufs=1) as cpool, \
             tc.tile_pool(name="xc", bufs=2) as xcp, \
             tc.tile_pool(name="qk16", bufs=3) as qk16p, \
             tc.tile_pool(name="rp", bufs=3) as rpp, \
             tc.tile_pool(name="scr", bufs=4) as scrp, \
             tc.tile_pool(name="ex", bufs=3) as exp_, \
             tc.tile_pool(name="ex8", bufs=3) as ex8p, \
             tc.tile_pool(name="lt", bufs=2) as ltp, \
             tc.tile_pool(name="bc", bufs=2) as bcp, \
             tc.tile_pool(name="yt", bufs=3) as ytp, \
             tc.tile_pool(name="prps", bufs=2, space="PSUM") as prps, \
             tc.tile_pool(name="scps", bufs=2, space="PSUM") as scps, \
             tc.tile_pool(name="pvps", bufs=2, space="PSUM") as pvps:

            wq8 = [cpool.tile([P, NKP, NDT, 2 * P], F8, tag=f"wq8{v}",
                              name=f"wq8{v}") for v in range(3)]
            wk8 = [cpool.tile([P, NKP, NDT, 2 * P], F8, tag=f"wk8{v}",
                              name=f"wk8{v}") for v in range(3)]
            wvt = cpool.tile([P, NKT, DG], F16, tag="wvt")
            wvr = cpool.tile([P, NKT, DG], F16, tag="wvr")
            wo_sb = cpool.tile([P, NDT, D], F16, tag="wo")
            cost = cpool.tile([P, S], F16, tag="cos")
            sint = cpool.tile([P, S], F16, tag="sin")
            attnT = cpool.tile([P, NDT, S], F16, tag="attnT")
            K16 = cpool.tile([P, NDT, S], F16, tag="k16")
            # fp8 interleaved V pairs (qb1-3), padded to M=128: flat col
            # 2*(127-m)+i -> dims at 128..255, ones (m=64) at 126..127,
            # zeros below (junk psum rows 65..127, never read)
            V8i = cpool.tile([P, NST // 2, HPC, 2 * P], F8, tag="v8i")
            # plain fp16 V for chunk 0 (q-block 0 runs fp16 PV)
            V16 = cpool.tile([P, 4, HPC, HD + 1], F16, tag="v16")
            ones = cpool.tile([P, NST * HPC], F32, tag="ones")

            nc.any.memset(ones[:], 1.0)
            nc.vector.tensor_copy(
                V16[:, :, :, HD:HD + 1],
                ones[:, 0:32].rearrange("p (a b) -> p a b", a=4),
            )
            nc.any.memset(V8i[:], 0.0)
            nc.vector.tensor_copy(
                V8i[:, :, :, 126:128],
                ones[:].rearrange("p (a b c) -> p a b c", a=NST // 2, b=HPC),
            )

            # ---- input loads (wk8 + x variants are the critical path) ----
            xv0 = [None, None, None]
            for v in range(3):
                xv0[v] = xcp.tile([P, NKT, CW], F8, tag=f"xv{v}", name=f"xv{v}0")
            xc0 = xcp.tile([P, NKT, CW], F16, tag="xc")
            for v in range(3):
                nc.sync.dma_start(out=wk8[v][:], in_=wk8_d[v].ap())
            nc.scalar.dma_start(out=xv0[0][:], in_=x8_d.ap()[:, 0])
            nc.gpsimd.dma_start(out=xv0[1][:], in_=xb_d.ap()[:, 0])
            nc.scalar.dma_start(out=xv0[2][:], in_=xc8_d.ap()[:, 0])
            nc.gpsimd.dma_start(out=xc0[:], in_=xT_v[:, 0])
            for v in range(3):
                nc.sync.dma_start(out=wq8[v][:], in_=wq8_d[v].ap())
            nc.sync.dma_start(out=wvr[:], in_=wvr_d.ap())
            nc.sync.dma_start(out=wvt[:, 0:2, :], in_=wv_d.ap()[:, 0:2, :])
            nc.sync.dma_start(out=wvt[:, 2:8, :], in_=wv_d.ap()[:, 2:8, :])
            nc.gpsimd.dma_start(out=cost[:], in_=cos_d.ap())
            nc.gpsimd.dma_start(out=sint[:], in_=sin_d.ap())
            nc.scalar.dma_start(out=wo_sb[:], in_=wo_d.ap())

            xcs = [xc0, None, None, None]
            xvs = [xv0, None, None, None]

            # ---------- phase-1 emitters ----------
            def proj_pair(xv, w8, dst16, c, a):
                """Project permA2 tiles 2a,2a+1 (residual fp8 SwInterleave:
                x8@w16 + (x/16)8@(16dw)8 + (4dx)8@(w16/4)8), rope, shuffle."""
                csl = slice(c * CW, (c + 1) * CW)
                t16 = qk16p.tile([P, 2, CW], F16, tag="t16")
                for e in range(2):
                    dt = 2 * a + e
                    psq = prps.tile([P, CW], F32, tag="pr")
                    first = True
                    for v in range(3):
                        for kp in range(NKP):
                            nc.tensor.matmul(
                                psq[:],
                                w8[v][:, kp, dt, :],
                                xv[v][:, 2 * kp:2 * kp + 2, :],
                                start=first,
                                stop=(v == 2 and kp == NKP - 1),
                                perf_mode=SWI,
                            )
                            first = False
                    nc.vector.tensor_copy(t16[:, e, :], psq[:])
                a0 = t16[:, 0, :]
                a1 = t16[:, 1, :]
                cc = cost[:, csl]
                ss = sint[:, csl]
                rp = rpp.tile([P, 2, CW], F16, tag="rp")
                m0 = scrp.tile([P, CW], F16, tag="m")
                m1 = scrp.tile([P, CW], F16, tag="m")
                nc.vector.tensor_mul(m0[:], a0, cc)
                nc.vector.tensor_mul(m1[:], a1, ss)
                nc.vector.tensor_sub(rp[:, 0, :], m0[:], m1[:])
                nc.vector.tensor_mul(m0[:], a0, ss)
                nc.vector.tensor_mul(m1[:], a1, cc)
                nc.vector.tensor_add(rp[:, 1, :], m0[:], m1[:])
                # shuffle: band m (head h=4a+m) -> head-contiguous dst16.
                # src iterates (i, e, s); dst rows 64*(h%2) + 2i+e = original
                # dim order.  Alternate dispatch queues to spread SEQ load.
                for m in range(4):
                    h = 4 * a + m
                    q = (nc.sync, nc.gpsimd)[m % 2]
                    q.dma_start(
                        out=dst16[64 * (h % 2):64 * (h % 2) + 64, h // 2, csl],
                        in_=rp[32 * m:32 * m + 32, :, :],
                    )

            def proj_v(xc, c, st):
                # fp8 interleaved V (host-reversed wv cols -> ascending
                # stride-2 writes): flat col 2j+2+i, i = key-tile parity
                psv = prps.tile([P, CW], F32, tag="pr")
                for kt in range(NKT):
                    nc.tensor.matmul(
                        psv[:],
                        xc[:, kt, st * P:(st + 1) * P],
                        wvr[:, kt, :],
                        start=(kt == 0),
                        stop=(kt == NKT - 1),
                    )
                jt = c * 4 + st
                nc.vector.tensor_copy(
                    V8i[:, jt // 2, :, (128 + jt % 2)::2],
                    psv[:].rearrange("p (h j) -> p h j", h=HPC),
                )
                if c == 0:
                    psn = prps.tile([P, CW], F32, tag="pr")
                    for kt in range(NKT):
                        nc.tensor.matmul(
                            psn[:],
                            xc[:, kt, st * P:(st + 1) * P],
                            wvt[:, kt, :],
                            start=(kt == 0),
                            stop=(kt == NKT - 1),
                        )
                    nc.vector.tensor_copy(
                        V16[:, st, :, 0:HD],
                        psn[:].rearrange("p (h d) -> p h d", h=HPC),
                    )

            Q16s = [None] * NSC

            def proj_units(c):
                xc = xcs[c]
                xv = xvs[c]
                q16 = qk16p.tile([P, NDT, CW], F16, tag="q16", bufs=2)
                Q16s[c] = q16
                units = []
                for a in range(2):
                    units.append(lambda a=a: proj_pair(xv, wk8, K16, c, a))
                for a in range(2):
                    units.append(
                        lambda a=a, q16=q16: proj_pair(
                            xv, wq8, _QView(q16), c, a
                        )
                    )
                for st in range(4):
                    units.append(lambda st=st: proj_v(xc, c, st))
                return units

            class _QView:
                """Adapter: Q16 is per-chunk [P, NDT, CW]; present the same
                indexing as K16 [P, NDT, S] (csl column slice)."""

                def __init__(self, t):
                    self.t = t

                def __getitem__(self, key):
                    rows, tl, csl = key
                    c0 = csl.start % CW
                    return self.t[rows, tl, c0:c0 + (csl.stop - csl.start)]

            # ---------- phase-2 emitter ----------
            def attn_head(qb, h, after_pair=None):
                rb = 64 * (h % 2)
                T = h // 2
                njt = 4 * (qb + 1)
                q16 = Q16s[qb]
                pso = pvps.tile([P, CW], F32, tag="pso")
                for jp in range(njt // 2):
                    pss = scps.tile([P, 2, CW], F32, tag="pss")
                    pqlo = max(0, 256 * jp - 512 * qb)
                    for jj in range(2):
                        jt = 2 * jp + jj
                        qlo = max(0, 128 * jt - 512 * qb)
                        nc.tensor.matmul(
                            pss[:, jj, qlo:CW],
                            K16[rb:rb + 64, T, jt * P:(jt + 1) * P],
                            q16[rb:rb + 64, T, qlo:CW],
                            start=True,
                            stop=True,
                        )
                    fp8 = qb >= 1
                    ex = (ex8p if fp8 else exp_).tile(
                        [P, 2, CW], F8 if fp8 else F16, tag="ex")
                    nc.scalar.activation(
                        ex[:, :, pqlo:CW],
                        pss[:, :, pqlo:CW],
                        AF.Exp,
                        scale=float(1.0 / (np.sqrt(HD) * WS * WS)),
                    )
                    jt0 = 2 * jp
                    if 128 * (jt0 + 1) >= 512 * qb:  # diagonal pair: mask both
                        nc.gpsimd.affine_select(
                            out=ex[:, :, pqlo:pqlo + 2 * P],
                            in_=ex[:, :, pqlo:pqlo + 2 * P],
                            compare_op=mybir.AluOpType.is_ge,
                            fill=0.0,
                            base=0,
                            pattern=[[-P, 2], [1, 2 * P]],
                            channel_multiplier=-1,
                        )
                    if fp8:
                        nc.tensor.matmul(
                            pso[:, pqlo:CW],
                            V8i[:, jp, h, :],
                            ex[:, :, pqlo:CW],
                            start=(jp == 0),
                            stop=(jp == njt // 2 - 1),
                            perf_mode=SWI,
                        )
                    else:
                        for jj in range(2):
                            jt = 2 * jp + jj
                            qlo = max(0, 128 * jt - 512 * qb)
                            nc.tensor.matmul(
                                pso[0:HD + 1, qlo:CW],
                                V16[:, jt, h, :],
                                ex[:, jj, qlo:CW],
                                start=(jt == 0),
                                stop=(jt == njt - 1),
                            )
                    if after_pair is not None:
                        after_pair()
                # normalization: pso rows = (d0..d63, l)
                qsl = slice(qb * CW, (qb + 1) * CW)
                pb = 64 * (h % 2)
                lt = ltp.tile([P, CW], F16, tag="lt")
                with nc.allow_low_precision(reason="1/l fp16 ok for 2e-2 tol"):
                    nc.vector.reciprocal(lt[HD:HD + 1, :], pso[HD:HD + 1, :])
                bct = bcp.tile([P, CW], F16, tag="bc")
                nc.sync.dma_start(
                    out=bct[:],
                    in_=lt[HD:HD + 1, :].unsqueeze(1).broadcast_to((1, P, CW)),
                )
                if h % 2 == 0:
                    nc.vector.tensor_mul(
                        attnT[0:HD, h // 2, qsl], pso[0:HD, :], bct[0:HD, :]
                    )
                else:
                    nc.vector.tensor_copy(attnT[pb:pb + HD, h // 2, qsl], pso[0:HD, :])
                    nc.vector.tensor_mul(
                        attnT[pb:pb + HD, h // 2, qsl],
                        attnT[pb:pb + HD, h // 2, qsl],
                        bct[pb:pb + HD, :],
                    )

            # ---------- phase-3 emitter: one-matmul steps for sprinkling ----
            def wo_half_steps(qt, nt):
                psy = prps.tile([P, CW], F32, tag="pr")
                for dt in range(NDT):
                    nc.tensor.matmul(
                        psy[:],
                        attnT[:, dt, qt * P:(qt + 1) * P],
                        wo_sb[:, dt, nt * CW:(nt + 1) * CW],
                        start=(dt == 0),
                        stop=(dt == NDT - 1),
                    )
                    yield
                yt = ytp.tile([P, CW], F16, tag="yt")
                nc.vector.tensor_copy(yt[:], psy[:])
                nc.scalar.dma_start(
                    out=y_d.ap()[qt * P:(qt + 1) * P, nt * CW:(nt + 1) * CW],
                    in_=yt[:],
                )
                yield

            def wo_stream(qtiles):
                for qt in qtiles:
                    for nt in range(2):
                        yield from wo_half_steps(qt, nt)

            def zip_emit(heads, units):
                ui = 0
                for i, hfn in enumerate(heads):
                    hfn()
                    tgt = ((i + 1) * len(units)) // max(1, len(heads))
                    while ui < tgt:
                        units[ui]()
                        ui += 1
                while ui < len(units):
                    units[ui]()
                    ui += 1

            # ---------- schedule ----------
            for u in proj_units(0):
                u()
            for c in range(1, NSC):
                xc = xcp.tile([P, NKT, CW], F16, tag="xc")
                nc.gpsimd.dma_start(out=xc[:], in_=xT_v[:, c])
                xcs[c] = xc
                xv = [None, None, None]
                for v, (d, q) in enumerate(
                        [(x8_d, nc.sync), (xb_d, nc.sync), (xc8_d, nc.scalar)]):
                    xv[v] = xcp.tile([P, NKT, CW], F8, tag=f"xv{v}",
                                     name=f"xv{v}{c}")
                    q.dma_start(out=xv[v][:], in_=d.ap()[:, c])
                xvs[c] = xv
                zip_emit([lambda h=h: attn_head(c - 1, h) for h in range(HPC)],
                         proj_units(c))
            # qb3 + wo sprinkled one matmul/evict at a time
            wo_gen = wo_stream(range(12))
            nsteps = 12 * 2 * (NDT + 1)
            npairs = HPC * NSC * 2 // 2
            state = {"frac": 0.0}

            def fill_wo():
                state["frac"] += nsteps / npairs
                while state["frac"] >= 1.0:
                    if next(wo_gen, "done") == "done":
                        state["frac"] = 0.0
                        return
                    state["frac"] -= 1.0

            for h in range(HPC):
                attn_head(NSC - 1, h, after_pair=fill_wo)
            for _ in wo_gen:
                pass
            for _ in wo_stream(range(12, 16)):
                pass

    nc.compile()
    return nc


def _perm_a2():
    """Column permutation for wq/wk: permuted col n = dt*128 + p with
    dt = 2*(h//4) + parity, p = 32*(h%4) + i  ->  original col h*64+2i+parity.
    """
    perm = np.empty(DG, dtype=np.int64)
    for n in range(DG):
        dt, p = n // P, n % P
        a, e = dt // 2, dt % 2
        m, i = p // 32, p % 32
        perm[n] = (4 * a + m) * HD + 2 * i + e
    return perm


def _pack_w(w):
    """[D, M] -> tile layout [P, NKT, M]."""
    return np.ascontiguousarray(
        w.reshape(NKT, P, -1).transpose(1, 0, 2)).astype(np.float16)


def _q8(a):
    import ml_dtypes
    return a.astype(ml_dtypes.float8_e4m3)


def _pack_w8i(wf):
    """[D, DG] fp32 (already perm'd+scaled) -> 3 interleaved fp8 variants
    [P, NKP, NDT, 256]: content[p, kp, dt, c] =
    Wv[(2kp + c%2)*128 + p, dt*128 + (127 - c//2)]."""
    w16_8 = _q8(wf)
    dw = wf - w16_8.astype(np.float32)
    variants = [w16_8, _q8(16.0 * dw), _q8(w16_8.astype(np.float32) / 4.0)]
    out = []
    cs = np.arange(2 * P)
    rows_i = cs % 2                      # k-tile parity per flat col
    cols_m = 127 - cs // 2               # output column per flat col
    for wv in variants:
        a = wv.reshape(NKT, P, NDT, P)   # [kt, p, dt, m]
        r = np.empty((P, NKP, NDT, 2 * P), dtype=wv.dtype)
        for kp in range(NKP):
            # [p, dt, c]
            r[:, kp] = a[2 * kp + rows_i, :, :, cols_m].transpose(1, 2, 0)
        out.append(np.ascontiguousarray(r))
    return out


def kernel(**inputs):
    global _PROGRAM
    x = np.asarray(inputs["x"], dtype=np.float32)
    freqs_cos = np.asarray(inputs["freqs_cos"], dtype=np.float32)
    freqs_sin = np.asarray(inputs["freqs_sin"], dtype=np.float32)
    wq = np.asarray(inputs["wq"], dtype=np.float32)
    wk = np.asarray(inputs["wk"], dtype=np.float32)
    wv = np.asarray(inputs["wv"], dtype=np.float32)
    wo = np.asarray(inputs["wo"], dtype=np.float32)

    if _PROGRAM is None:
        _PROGRAM = _build_program()
    nc = _PROGRAM

    perm = _perm_a2()
    cost = np.ascontiguousarray(np.tile(freqs_cos.T, (4, 1))).astype(np.float16)
    sint = np.ascontiguousarray(np.tile(freqs_sin.T, (4, 1))).astype(np.float16)
    # per-head reversed wv columns (for the interleaved fp8 V layout)
    rperm = np.arange(DG).reshape(HPC, HD)[:, ::-1].reshape(-1)

    in_maps = []
    for c in range(NCORES):
        b, g = c // 2, c % 2
        gsl = slice(g * DG, (g + 1) * DG)
        xt = x[b].T.reshape(NKT, P, NSC, CW).transpose(1, 2, 0, 3)
        xt = np.ascontiguousarray(xt)
        x8 = _q8(xt)
        dx = xt - x8.astype(np.float32)
        wq8v = _pack_w8i(WS * wq[:, gsl][:, perm])
        wk8v = _pack_w8i(WS * wk[:, gsl][:, perm])
        m = {
            "xT": xt.astype(np.float16),
            "x8": x8,
            "xb": _q8(xt / 16.0),
            "xc8": _q8(4.0 * dx),
            "wv": _pack_w(wv[:, gsl]),
            "wvr": _pack_w(wv[:, gsl][:, rperm]),
            "wo": np.ascontiguousarray(
                wo[gsl, :].reshape(NDT, P, D).transpose(1, 0, 2)
            ).astype(np.float16),
            "cost": cost,
            "sint": sint,
        }
        for v in range(3):
            m[f"wq8{v}"] = wq8v[v]
            m[f"wk8{v}"] = wk8v[v]
        in_maps.append(m)

    res = run_bass_kernel_spmd(nc, in_maps, list(range(NCORES)))
    y = np.empty((B, S, D), dtype=np.float32)
    for b in range(B):
        y[b] = (res.results[2 * b]["y"].astype(np.float32)
                + res.results[2 * b + 1]["y"].astype(np.float32))
    return y


# revision 3
# speedup vs baseline: 1.0279x; 1.0279x over previous
"""Trainium2 Bass kernel v4 for nn_Attention_36137854828870.

Multi-head causal attention with rotary embeddings:
  y = softmax((rope(x@wq) @ rope(x@wk)^T)/sqrt(hd) + causal) @ (x@wv) @ wo

Sharding (8 cores): data-parallel over batch (4) x tensor-parallel over
heads (2 groups of 8).  Core c handles batch c//2, head group c%2; host
sums the two partial (S, D) outputs per batch.

Everything runs in fp16 on the PE (1 cycle/column; fp8 DoubleRow would
halve this but is TRN3-only silicon).  Per-core pipeline, all on-chip:

  - Projections stream in 4 s-chunks of 512.  wq/wk columns are
    host-permuted ("permA2") so rope pairs are lane-aligned: head h of
    the group lives in partition band 32*(h%4)..+32 of partition-tile
    pair (2*(h//4), 2*(h//4)+1), even rope-dims in the even tile, odd
    dims in the odd tile.  Rope runs on DVE in fp16 2x mode.
  - A small SBUF->SBUF DMA shuffle (8 DMAs per tensor-chunk, 64 total)
    rearranges rope output into head-contiguous K16/Q16 (head h = 64
    rows at 64*(h%2), tile h//2, dims in original order).  No DRAM
    staging roundtrip, no gpsimd SWDGE storm.
  - Scores: fp16 64-partition matmuls, keys on PSUM partitions, q on
    free dim, emitted per key-tile pair with per-tile causal trimming.
  - exp on ScalarE once per key-tile pair ([128, 2, cols] PSUM->SBUF),
    scale=1/sqrt(hd) folded in, no max-subtraction (|scores| <= ~4).
    Causal triangles masked with one 2-D-pattern gpsimd.affine_select
    per diagonal pair, off the exp->PV critical path.
  - PV: fp16 per key-tile, accumulating [65, cols] PSUM (64 head dims
    + a ones-column denominator row riding in V).
  - Deferred normalization: reciprocal of the denominator row, 0-stride
    broadcast DMA, fused multiply-evict (even heads) or the
    baseline-proven shift-copy + in-place multiply (odd heads).
  - wo in fp16 with q on PSUM partitions; fp16 y out (host upcasts and
    sums the two per-batch partials).

Emission interleaves chunk-c projections with q-block c-1 attention so
the ScalarE exp stream never starves behind PE's projection stream; wo
matmuls are sprinkled one at a time between q-block-3 attention pairs
to fill PE while ScalarE finishes the last exps.
"""

import sys

sys.path.insert(0, "/opt/trn_rl_repo")

import numpy as np

import concourse.bass as bass
import concourse.mybir as mybir
import concourse.tile as tile
from concourse import bacc
from concourse.bass_utils import run_bass_kernel_spmd

B, S, D = 4, 2048, 1024
H, HD = 16, 64
P = 128
NCORES = 8
HPC = H // 2          # heads per core
DG = HPC * HD         # 512
NKT = D // P          # 8
NDT = DG // P         # 4
CW = 512              # s-chunk / q-block width
NSC = S // CW         # 4
NST = S // P          # 16 key tiles
F32 = mybir.dt.float32
F16 = mybir.dt.float16
F8 = mybir.dt.float8e4
AF = mybir.ActivationFunctionType
SWI = mybir.MatmulPerfMode.DoubleRowSwInterleave
WS = 16.0             # host scale on wq/wk (fp8 denormal avoidance)
NKP = NKT // 2        # 4 contraction k-tile pairs

_PROGRAM = None


def _build_program():
    nc = bacc.Bacc("TRN2", target_bir_lowering=False, debug=False)

    # all inputs pre-packed on host into SBUF tile layout (fully
    # contiguous per DMA -> no small-descriptor penalty)
    xT_d = nc.dram_tensor("xT", [P, NSC, NKT, CW], F16, kind="ExternalInput")
    # fp8 x variants for the residual projection (a: x8, b: (x/16)8, c: (4dx)8)
    x8_d = nc.dram_tensor("x8", [P, NSC, NKT, CW], F8, kind="ExternalInput")
    xb_d = nc.dram_tensor("xb", [P, NSC, NKT, CW], F8, kind="ExternalInput")
    xc8_d = nc.dram_tensor("xc8", [P, NSC, NKT, CW], F8, kind="ExternalInput")
    # interleaved fp8 weight variants [P, NKP, NDT, 256]
    wq8_d = [nc.dram_tensor(f"wq8{v}", [P, NKP, NDT, 2 * P], F8,
                            kind="ExternalInput") for v in range(3)]
    wk8_d = [nc.dram_tensor(f"wk8{v}", [P, NKP, NDT, 2 * P], F8,
                            kind="ExternalInput") for v in range(3)]
    wv_d = nc.dram_tensor("wv", [P, NKT, DG], F16, kind="ExternalInput")
    wvr_d = nc.dram_tensor("wvr", [P, NKT, DG], F16, kind="ExternalInput")
    wo_d = nc.dram_tensor("wo", [P, NDT, D], F16, kind="ExternalInput")
    cos_d = nc.dram_tensor("cost", [P, S], F16, kind="ExternalInput")
    sin_d = nc.dram_tensor("sint", [P, S], F16, kind="ExternalInput")
    y_d = nc.dram_tensor("y", [S, D], F16, kind="ExternalOutput")

    xT_v = xT_d.ap()

    with tile.TileContext(nc) as tc:
        with tc.tile_pool(name="const", bufs=1) as cpool, \
             tc.tile_pool(name="xc", bufs=2) as xcp, \
             tc.tile_pool(name="qk16", bufs=4) as qk16p, \
             tc.tile_pool(name="rp", bufs=4) as rpp, \
             tc.tile_pool(name="scr", bufs=6) as scrp, \
             tc.tile_pool(name="ex", bufs=4) as exp_, \
             tc.tile_pool(name="ex8", bufs=5) as ex8p, \
             tc.tile_pool(name="lt", bufs=3) as ltp, \
             tc.tile_pool(name="bc", bufs=3) as bcp, \
             tc.tile_pool(name="yt", bufs=4) as ytp, \
             tc.tile_pool(name="prps", bufs=2, space="PSUM") as prps, \
             tc.tile_pool(name="scps", bufs=2, space="PSUM") as scps, \
             tc.tile_pool(name="pvps", bufs=2, space="PSUM") as pvps:

            wq8 = [cpool.tile([P, NKP, NDT, 2 * P], F8, tag=f"wq8{v}",
                              name=f"wq8{v}") for v in range(3)]
            wk8 = [cpool.tile([P, NKP, NDT, 2 * P], F8, tag=f"wk8{v}",
                              name=f"wk8{v}") for v in range(3)]
            wvt = cpool.tile([P, NKT, DG], F16, tag="wvt")
            wvr = cpool.tile([P, NKT, DG], F16, tag="wvr")
            wo_sb = cpool.tile([P, NDT, D], F16, tag="wo")
            cost = cpool.tile([P, S], F16, tag="cos")
            sint = cpool.tile([P, S], F16, tag="sin")
            attnT = cpool.tile([P, NDT, S], F16, tag="attnT")
            K16 = cpool.tile([P, NDT, S], F16, tag="k16")
            # fp8 interleaved V pairs (qb1-3), padded to M=128: flat col
            # 2*(127-m)+i -> dims at 128..255, ones (m=64) at 126..127,
            # zeros below (junk psum rows 65..127, never read)
            V8i = cpool.tile([P, NST // 2, HPC, 2 * P], F8, tag="v8i")
            # plain fp16 V for chunk 0 (q-block 0 runs fp16 PV)
            V16 = cpool.tile([P, 4, HPC, HD + 1], F16, tag="v16")
            ones = cpool.tile([P, NST * HPC], F32, tag="ones")

            nc.any.memset(ones[:], 1.0)
            nc.vector.tensor_copy(
                V16[:, :, :, HD:HD + 1],
                ones[:, 0:32].rearrange("p (a b) -> p a b", a=4),
            )
            nc.any.memset(V8i[:], 0.0)
            nc.vector.tensor_copy(
                V8i[:, :, :, 126:128],
                ones[:].rearrange("p (a b c) -> p a b c", a=NST // 2, b=HPC),
            )

            # ---- input loads (wk8 + x variants are the critical path) ----
            xv0 = [None, None, None]
            for v in range(3):
                xv0[v] = xcp.tile([P, NKT, CW], F8, tag=f"xv{v}", name=f"xv{v}0")
            xc0 = xcp.tile([P, NKT, CW], F16, tag="xc")
            for v in range(3):
                nc.sync.dma_start(out=wk8[v][:], in_=wk8_d[v].ap())
            nc.scalar.dma_start(out=xv0[0][:], in_=x8_d.ap()[:, 0])
            nc.gpsimd.dma_start(out=xv0[1][:], in_=xb_d.ap()[:, 0])
            nc.scalar.dma_start(out=xv0[2][:], in_=xc8_d.ap()[:, 0])
            nc.gpsimd.dma_start(out=xc0[:], in_=xT_v[:, 0])
            for v in range(3):
                nc.sync.dma_start(out=wq8[v][:], in_=wq8_d[v].ap())
            nc.sync.dma_start(out=wvr[:], in_=wvr_d.ap())
            nc.sync.dma_start(out=wvt[:, 0:2, :], in_=wv_d.ap()[:, 0:2, :])
            nc.sync.dma_start(out=wvt[:, 2:8, :], in_=wv_d.ap()[:, 2:8, :])
            nc.gpsimd.dma_start(out=cost[:], in_=cos_d.ap())
            nc.gpsimd.dma_start(out=sint[:], in_=sin_d.ap())
            nc.scalar.dma_start(out=wo_sb[:], in_=wo_d.ap())

            xcs = [xc0, None, None, None]
            xvs = [xv0, None, None, None]

            # ---------- phase-1 emitters ----------
            def proj_pair(xv, w8, dst16, c, a):
                """Project permA2 tiles 2a,2a+1 (residual fp8 SwInterleave:
                x8@w16 + (x/16)8@(16dw)8 + (4dx)8@(w16/4)8), rope, shuffle."""
                csl = slice(c * CW, (c + 1) * CW)
                t16 = qk16p.tile([P, 2, CW], F16, tag="t16")
                for e in range(2):
                    dt = 2 * a + e
                    psq = prps.tile([P, CW], F32, tag="pr")
                    first = True
                    for v in range(3):
                        for kp in range(NKP):
                            nc.tensor.matmul(
                                psq[:],
                                w8[v][:, kp, dt, :],
                                xv[v][:, 2 * kp:2 * kp + 2, :],
                                start=first,
                                stop=(v == 2 and kp == NKP - 1),
                                perf_mode=SWI,
                            )
                            first = False
                    nc.vector.tensor_copy(t16[:, e, :], psq[:])
                a0 = t16[:, 0, :]
                a1 = t16[:, 1, :]
                cc = cost[:, csl]
                ss = sint[:, csl]
                rp = rpp.tile([P, 2, CW], F16, tag="rp")
                m0 = scrp.tile([P, CW], F16, tag="m")
                m1 = scrp.tile([P, CW], F16, tag="m")
                nc.vector.tensor_mul(m0[:], a0, cc)
                nc.vector.tensor_mul(m1[:], a1, ss)
                nc.vector.tensor_sub(rp[:, 0, :], m0[:], m1[:])
                nc.vector.tensor_mul(m0[:], a0, ss)
                nc.vector.tensor_mul(m1[:], a1, cc)
                nc.vector.tensor_add(rp[:, 1, :], m0[:], m1[:])
                # shuffle: band m (head h=4a+m) -> head-contiguous dst16.
                # src iterates (i, e, s); dst rows 64*(h%2) + 2i+e = original
                # dim order.  Alternate dispatch queues to spread SEQ load.
                for m in range(4):
                    h = 4 * a + m
                    q = (nc.sync, nc.gpsimd)[m % 2]
                    q.dma_start(
                        out=dst16[64 * (h % 2):64 * (h % 2) + 64, h // 2, csl],
                        in_=rp[32 * m:32 * m + 32, :, :],
                    )

            def proj_v(xc, c, st):
                # fp8 interleaved V (host-reversed wv cols -> ascending
                # stride-2 writes): flat col 2j+2+i, i = key-tile parity
                psv = prps.tile([P, CW], F32, tag="pr")
                for kt in range(NKT):
                    nc.tensor.matmul(
                        psv[:],
                        xc[:, kt, st * P:(st + 1) * P],
                        wvr[:, kt, :],
                        start=(kt == 0),
                        stop=(kt == NKT - 1),
                    )
                jt = c * 4 + st
                nc.vector.tensor_copy(
                    V8i[:, jt // 2, :, (128 + jt % 2)::2],
                    psv[:].rearrange("p (h j) -> p h j", h=HPC),
                )
                if c == 0:
                    psn = prps.tile([P, CW], F32, tag="pr")
                    for kt in range(NKT):
                        nc.tensor.matmul(
                            psn[:],
                            xc[:, kt, st * P:(st + 1) * P],
                            wvt[:, kt, :],
                            start=(kt == 0),
                            stop=(kt == NKT - 1),
                        )
                    nc.vector.tensor_copy(
                        V16[:, st, :, 0:HD],
                        psn[:].rearrange("p (h d) -> p h d", h=HPC),
                    )

            Q16s = [None] * NSC

            def proj_units(c):
                xc = xcs[c]
                xv = xvs[c]
                q16 = qk16p.tile([P, NDT, CW], F16, tag="q16", bufs=2)
                Q16s[c] = q16
                units = []
                for a in range(2):
                    units.append(lambda a=a: proj_pair(xv, wk8, K16, c, a))
                for a in range(2):
                    units.append(
                        lambda a=a, q16=q16: proj_pair(
                            xv, wq8, _QView(q16), c, a
                        )
                    )
                for st in range(4):
                    units.append(lambda st=st: proj_v(xc, c, st))
                return units

            class _QView:
                """Adapter: Q16 is per-chunk [P, NDT, CW]; present the same
                indexing as K16 [P, NDT, S] (csl column slice)."""

                def __init__(self, t):
                    self.t = t

                def __getitem__(self, key):
                    rows, tl, csl = key
                    c0 = csl.start % CW
                    return self.t[rows, tl, c0:c0 + (csl.stop - csl.start)]

            # ---------- phase-2 emitter ----------
            def attn_head(qb, h, after_pair=None):
                rb = 64 * (h % 2)
                T = h // 2
                njt = 4 * (qb + 1)
                q16 = Q16s[qb]
                pso = pvps.tile([P, CW], F32, tag="pso")
                for jp in range(njt // 2):
                    pss = scps.tile([P, 2, CW], F32, tag="pss")
                    pqlo = max(0, 256 * jp - 512 * qb)
                    for jj in range(2):
                        jt = 2 * jp + jj
                        qlo = max(0, 128 * jt - 512 * qb)
                        nc.tensor.matmul(
                            pss[:, jj, qlo:CW],
                            K16[rb:rb + 64, T, jt * P:(jt + 1) * P],
                            q16[rb:rb + 64, T, qlo:CW],
                            start=True,
                            stop=True,
                        )
                    fp8 = qb >= 1
                    ex = (ex8p if fp8 else exp_).tile(
                        [P, 2, CW], F8 if fp8 else F16, tag="ex")
                    nc.scalar.activation(
                        ex[:, :, pqlo:CW],
                        pss[:, :, pqlo:CW],
                        AF.Exp,
                        scale=float(1.0 / (np.sqrt(HD) * WS * WS)),
                    )
                    jt0 = 2 * jp
                    if 128 * (jt0 + 1) >= 512 * qb:  # diagonal pair: mask both
                        nc.gpsimd.affine_select(
                            out=ex[:, :, pqlo:pqlo + 2 * P],
                            in_=ex[:, :, pqlo:pqlo + 2 * P],
                            compare_op=mybir.AluOpType.is_ge,
                            fill=0.0,
                            base=0,
                            pattern=[[-P, 2], [1, 2 * P]],
                            channel_multiplier=-1,
                        )
                    if fp8:
                        nc.tensor.matmul(
                            pso[:, pqlo:CW],
                            V8i[:, jp, h, :],
                            ex[:, :, pqlo:CW],
                            start=(jp == 0),
                            stop=(jp == njt // 2 - 1),
                            perf_mode=SWI,
                        )
                    else:
                        for jj in range(2):
                            jt = 2 * jp + jj
                            qlo = max(0, 128 * jt - 512 * qb)
                            nc.tensor.matmul(
                                pso[0:HD + 1, qlo:CW],
                                V16[:, jt, h, :],
                                ex[:, jj, qlo:CW],
                                start=(jt == 0),
                                stop=(jt == njt - 1),
                            )
                    if after_pair is not None:
                        after_pair()
                # normalization: pso rows = (d0..d63, l)
                qsl = slice(qb * CW, (qb + 1) * CW)
                pb = 64 * (h % 2)
                lt = ltp.tile([P, CW], F16, tag="lt")
                with nc.allow_low_precision(reason="1/l fp16 ok for 2e-2 tol"):
                    nc.vector.reciprocal(lt[HD:HD + 1, :], pso[HD:HD + 1, :])
                bct = bcp.tile([P, CW], F16, tag="bc")
                nc.sync.dma_start(
                    out=bct[:],
                    in_=lt[HD:HD + 1, :].unsqueeze(1).broadcast_to((1, P, CW)),
                )
                if h % 2 == 0:
                    nc.vector.tensor_mul(
                        attnT[0:HD, h // 2, qsl], pso[0:HD, :], bct[0:HD, :]
                    )
                else:
                    nc.vector.tensor_copy(attnT[pb:pb + HD, h // 2, qsl], pso[0:HD, :])
                    nc.vector.tensor_mul(
                        attnT[pb:pb + HD, h // 2, qsl],
                        attnT[pb:pb + HD, h // 2, qsl],
                        bct[pb:pb + HD, :],
                    )

            # ---------- phase-3 emitter: one-matmul steps for sprinkling ----
            def wo_half_steps(qt, nt):
                psy = prps.tile([P, CW], F32, tag="pr")
                for dt in range(NDT):
                    nc.tensor.matmul(
                        psy[:],
                        attnT[:, dt, qt * P:(qt + 1) * P],
                        wo_sb[:, dt, nt * CW:(nt + 1) * CW],
                        start=(dt == 0),
                        stop=(dt == NDT - 1),
                    )
                    yield
                yt = ytp.tile([P, CW], F16, tag="yt")
                nc.vector.tensor_copy(yt[:], psy[:])
                nc.scalar.dma_start(
                    out=y_d.ap()[qt * P:(qt + 1) * P, nt * CW:(nt + 1) * CW],
                    in_=yt[:],
                )
                yield

            def wo_stream(qtiles):
                for qt in qtiles:
                    for nt in range(2):
                        yield from wo_half_steps(qt, nt)

            def zip_emit(heads, units):
                ui = 0
                for i, hfn in enumerate(heads):
                    hfn()
                    tgt = ((i + 1) * len(units)) // max(1, len(heads))
                    while ui < tgt:
                        units[ui]()
                        ui += 1
                while ui < len(units):
                    units[ui]()
                    ui += 1

            # ---------- schedule ----------
            for u in proj_units(0):
                u()
            for c in range(1, NSC):
                xc = xcp.tile([P, NKT, CW], F16, tag="xc")
                nc.gpsimd.dma_start(out=xc[:], in_=xT_v[:, c])
                xcs[c] = xc
                xv = [None, None, None]
                for v, (d, q) in enumerate(
                        [(x8_d, nc.sync), (xb_d, nc.sync), (xc8_d, nc.scalar)]):
                    xv[v] = xcp.tile([P, NKT, CW], F8, tag=f"xv{v}",
                                     name=f"xv{v}{c}")
                    q.dma_start(out=xv[v][:], in_=d.ap()[:, c])
                xvs[c] = xv
                zip_emit([lambda h=h: attn_head(c - 1, h) for h in range(HPC)],
                         proj_units(c))
            # qb3 + wo sprinkled one matmul/evict at a time
            wo_gen = wo_stream(range(12))
            nsteps = 12 * 2 * (NDT + 1)
            npairs = HPC * NSC * 2 // 2
            state = {"frac": 0.0}

            def fill_wo():
                state["frac"] += nsteps / npairs
                while state["frac"] >= 1.0:
                    if next(wo_gen, "done") == "done":
                        state["frac"] = 0.0
                        return
                    state["frac"] -= 1.0

            for h in range(HPC):
                attn_head(NSC - 1, h, after_pair=fill_wo)
            for _ in wo_gen:
                pass
            for _ in wo_stream(range(12, 16)):
                pass

    nc.compile()
    return nc


def _perm_a2():
    """Column permutation for wq/wk: permuted col n = dt*128 + p with
    dt = 2*(h//4) + parity, p = 32*(h%4) + i  ->  original col h*64+2i+parity.
    """
    perm = np.empty(DG, dtype=np.int64)
    for n in range(DG):
        dt, p = n // P, n % P
        a, e = dt // 2, dt % 2
        m, i = p // 32, p % 32
        perm[n] = (4 * a + m) * HD + 2 * i + e
    return perm


def _pack_w(w):
    """[D, M] -> tile layout [P, NKT, M]."""
    return np.ascontiguousarray(
        w.reshape(NKT, P, -1).transpose(1, 0, 2)).astype(np.float16)


def _q8(a):
    import ml_dtypes
    return a.astype(ml_dtypes.float8_e4m3)


def _pack_w8i(wf):
    """[D, DG] fp32 (already perm'd+scaled) -> 3 interleaved fp8 variants
    [P, NKP, NDT, 256]: content[p, kp, dt, c] =
    Wv[(2kp + c%2)*128 + p, dt*128 + (127 - c//2)]."""
    w16_8 = _q8(wf)
    dw = wf - w16_8.astype(np.float32)
    variants = [w16_8, _q8(16.0 * dw), _q8(w16_8.astype(np.float32) / 4.0)]
    out = []
    cs = np.arange(2 * P)
    rows_i = cs % 2                      # k-tile parity per flat col
    cols_m = 127 - cs // 2               # output column per flat col
    for wv in variants:
        a = wv.reshape(NKT, P, NDT, P)   # [kt, p, dt, m]
        r = np.empty((P, NKP, NDT, 2 * P), dtype=wv.dtype)
        for kp in range(NKP):
            # [p, dt, c]
            r[:, kp] = a[2 * kp + rows_i, :, :, cols_m].transpose(1, 2, 0)
        out.append(np.ascontiguousarray(r))
    return out


def kernel(**inputs):
    global _PROGRAM
    x = np.asarray(inputs["x"], dtype=np.float32)
    freqs_cos = np.asarray(inputs["freqs_cos"], dtype=np.float32)
    freqs_sin = np.asarray(inputs["freqs_sin"], dtype=np.float32)
    wq = np.asarray(inputs["wq"], dtype=np.float32)
    wk = np.asarray(inputs["wk"], dtype=np.float32)
    wv = np.asarray(inputs["wv"], dtype=np.float32)
    wo = np.asarray(inputs["wo"], dtype=np.float32)

    if _PROGRAM is None:
        _PROGRAM = _build_program()
    nc = _PROGRAM

    perm = _perm_a2()
    cost = np.ascontiguousarray(np.tile(freqs_cos.T, (4, 1))).astype(np.float16)
    sint = np.ascontiguousarray(np.tile(freqs_sin.T, (4, 1))).astype(np.float16)
    # per-head reversed wv columns (for the interleaved fp8 V layout)
    rperm = np.arange(DG).reshape(HPC, HD)[:, ::-1].reshape(-1)

    in_maps = []
    for c in range(NCORES):
        b, g = c // 2, c % 2
        gsl = slice(g * DG, (g + 1) * DG)
        xt = x[b].T.reshape(NKT, P, NSC, CW).transpose(1, 2, 0, 3)
        xt = np.ascontiguousarray(xt)
        x8 = _q8(xt)
        dx = xt - x8.astype(np.float32)
        wq8v = _pack_w8i(WS * wq[:, gsl][:, perm])
        wk8v = _pack_w8i(WS * wk[:, gsl][:, perm])
        m = {
            "xT": xt.astype(np.float16),
            "x8": x8,
            "xb": _q8(xt / 16.0),
            "xc8": _q8(4.0 * dx),
            "wv": _pack_w(wv[:, gsl]),
            "wvr": _pack_w(wv[:, gsl][:, rperm]),
            "wo": np.ascontiguousarray(
                wo[gsl, :].reshape(NDT, P, D).transpose(1, 0, 2)
            ).astype(np.float16),
            "cost": cost,
            "sint": sint,
        }
        for v in range(3):
            m[f"wq8{v}"] = wq8v[v]
            m[f"wk8{v}"] = wk8v[v]
        in_maps.append(m)

    res = run_bass_kernel_spmd(nc, in_maps, list(range(NCORES)))
    y = np.empty((B, S, D), dtype=np.float32)
    for b in range(B):
        y[b] = (res.results[2 * b]["y"].astype(np.float32)
                + res.results[2 * b + 1]["y"].astype(np.float32))
    return y
